# revision 1
# baseline (speedup 1.0000x reference)
"""AudioAttention forward on 8 Trainium2 NeuronCores (Bass/Tile).

Reference computation (eval-mode AudioAttention):
    z      = mean_pool(Z_img)                    # [B, C]
    z_img, query = z[:, :C-A], z[:, C-A:]
    snd    = Z_snd[pad_idx]                      # [G, S, C] ragged gather
    value, key = snd[..., :C-A], snd[..., C-A:]
    scores = query @ key^T  (per group), masked softmax over S
    M_snd  = attn @ value                        # [G, B, C-A]
    M_img  = broadcast(z_img)                    # [G, B, C-A]

Sharding: groups are sorted by size and dealt round-robin to the 8
cores, so every core gets the same per-slot capacity profile -> one
SPMD program serves all cores (only the DRAM contents differ).
Capacities are rounded up to multiples of 128 so every 128-token chunk
is partition-aligned, which lets the whole key/value arrays live in
SBUF and stream in via a handful of large DMAs (the DMA *instruction*
dispatch on the queue engine costs ~0.8us each, so instruction count
matters more than bytes).

Device kernel, per 128-token chunk k of group slot j:
  scoresT [128,B] = matmul(lhsT=keyT_ext[65, 128], rhs=qT_ext[65, B])
      where row 64 of keyT_ext carries (-shift) for valid tokens and
      -30000 for padding, and row 64 of qT_ext is ones -> the mask and
      the softmax shift are folded into the contraction for free
      (exp(-30000) == 0 exactly, so pad tokens vanish).
  attnT = exp(scoresT)              (ACT engine, PSUM -> SBUF)
  m_j [B, 450] += matmul(lhsT=attnT[128, B], rhs=val_ext[128, 450])
      where val_ext column 448 is 1.0 for valid rows -> column 448
      accumulates the softmax denominator (449 is zero padding: the
      fp32r matmul requires an even destination free size).
  out_j = m_j[:, :448] * reciprocal(m_j[:, 448])  (DVE), one final DMA.

Matmuls run as float32r (TF32): same bits as fp32, 4x the fp32 PE
streaming rate. The softmax shift per group is a Cauchy-Schwarz upper
bound on the scores (max_b |q_b| * max_s |k_s|), so exp never
overflows; softmax is shift-invariant so the result is exact.
"""

import sys

if "/opt/trn_rl_repo" not in sys.path:
    sys.path.insert(0, "/opt/trn_rl_repo")

import numpy as np

N_CORES = 8
CHUNK = 128
N_SPLIT_V = 11  # value-array DMA split (parallel queues + early compute start)
N_SPLIT_K = 4  # key-array DMA split

LAST_RESULTS = None  # BassKernelResults of the most recent run (for test harness)


def _build_program(caps, gpc, ca):
    """One Bass program shared by all 8 cores.

    caps: per-slot token capacities, multiples of 128 (same on every core).
    gpc:  groups (slots) per core.
    ca:   C - A (value feature width).
    """
    from concourse import bacc, mybir
    from concourse.tile import TileContext

    vw = ca + 2  # value row width: features + denominator column + pad
    n_chunks = int(sum(caps)) // CHUNK
    sum_caps = n_chunks * CHUNK
    nc = bacc.Bacc(None, target_bir_lowering=False, debug=False)

    f32 = mybir.dt.float32
    f32r = mybir.dt.float32r
    f16 = mybir.dt.float16
    keys_d = nc.dram_tensor("keysT", [65, sum_caps], f16, kind="ExternalInput")
    vals_d = nc.dram_tensor("vals", [CHUNK, n_chunks * vw], f16, kind="ExternalInput")
    qt_d = nc.dram_tensor("qT", [65, 16], f16, kind="ExternalInput")
    out_d = nc.dram_tensor("out", [16, gpc * ca], f16, kind="ExternalOutput")


    def split(n, parts):
        q, r = divmod(n, parts)
        out, a = [], 0
        for i in range(parts):
            b = a + q + (1 if i < r else 0)
            if b > a:
                out.append((a, b))
            a = b
        return out

    with TileContext(nc) as tc:
        with (
            tc.tile_pool(name="resid", bufs=1) as rpool,
            tc.tile_pool(name="attn", bufs=3) as apool,
            tc.tile_pool(name="recp", bufs=4) as recpool,
            tc.tile_pool(name="scps", bufs=2, space="PSUM") as scpsum,
            tc.tile_pool(name="mps", bufs=4, space="PSUM") as mpsum,
            tc.tile_pool(name="wps", bufs=1, space="PSUM") as wpsum,
        ):
            qt = rpool.tile([65, 16], f16)
            nc.sync.dma_start(out=qt[:], in_=qt_d[:])
            ktile = rpool.tile([65, sum_caps], f16)
            vtile = rpool.tile([CHUNK, n_chunks * vw], f16)
            # All keys load FIRST (small array; the scores->exp chain gates
            # the whole pipeline), then values stream behind them. Only the
            # two HWDGE channels (Sync + Scalar/ACT) carry DMAs: involving
    	    # GpSimd (SWDGE) costs a ~4us queue drain in the kernel tail.
            dma_engines = [nc.sync, nc.scalar]

            def head_parts(n, head, parts):
                out = [(0, min(head, n))]
                if n > head:
                    out += [(a + head, b + head) for a, b in split(n - head, parts)]
                return out

            # One whole-array key DMA: the 65-partition strided pattern runs
            # ~50 GB/s, so long contiguous lines beat pipelined splits.
            nc.scalar.dma_start(out=ktile[:], in_=keys_d[:])
            for i, (a, b) in enumerate(head_parts(n_chunks, 4, N_SPLIT_V - 1)):
                dma_engines[i % 2].dma_start(
                    out=vtile[:, a * vw : b * vw], in_=vals_d[:, a * vw : b * vw]
                )
            obuf = rpool.tile([16, gpc * ca], f16)

            bf16 = mybir.dt.bfloat16
            warm = rpool.tile([CHUNK, 512], bf16)
            nc.vector.memset(warm[:], 0.0)
            wps = wpsum.tile([CHUNK, 512], f32)
            for _ in range(10):
                nc.tensor.matmul(wps[:], warm[:, :CHUNK], warm[:], start=True, stop=True)

            # Scores + exp are per-token, so batch GEXP chunks (across slot
            # boundaries) into one PSUM bank and one ACT exp instruction.
            GEXP = 8
            attn_tiles = {}

            def ensure_attn(kk):
                gi = kk // GEXP
                if gi in attn_tiles:
                    return attn_tiles[gi]
                n = min(GEXP, n_chunks - gi * GEXP)
                sc = scpsum.tile([CHUNK, n * 16], f32, name=f"sc{gi}", tag="sc")
                for x in range(n):
                    t0 = (gi * GEXP + x) * CHUNK
                    nc.tensor.matmul(
                        sc[:, x * 16 : (x + 1) * 16],
                        ktile[:, t0 : t0 + CHUNK],
                        qt[:],
                        start=True,
                        stop=True,
                    )
                at = apool.tile([CHUNK, n * 16], f16, name=f"at{gi}", tag="a")
                nc.scalar.activation(at[:], sc[:], mybir.ActivationFunctionType.Exp)
                attn_tiles[gi] = at
                return at

            k = 0
            for j in range(gpc):
                nck = int(caps[j]) // CHUNK
                m = mpsum.tile([16, vw], f32, name=f"m{j}", tag="m")
                for ci in range(nck):
                    kk = k + ci
                    at = ensure_attn(kk)
                    x = kk % GEXP
                    nc.tensor.matmul(
                        m[:],
                        at[:, x * 16 : (x + 1) * 16],
                        vtile[:, kk * vw : (kk + 1) * vw],
                        start=(ci == 0),
                        stop=(ci == nck - 1),
                    )
                rec = recpool.tile([16, 1], f32, name=f"r{j}", tag="r")
                nc.vector.reciprocal(rec[:], m[:, ca : ca + 1])
                if j % 2 == 0:
                    nc.vector.tensor_scalar_mul(
                        obuf[:, j * ca : (j + 1) * ca], m[:, 0:ca], rec[:]
                    )
                else:
                    nc.scalar.activation(
                        obuf[:, j * ca : (j + 1) * ca],
                        m[:, 0:ca],
                        mybir.ActivationFunctionType.Copy,
                        scale=rec[:],
                    )
                k += nck

            for i, (a, b) in enumerate(split(gpc, 4)):
                dma_engines[i % len(dma_engines)].dma_start(
                    out=out_d[:, a * ca : b * ca], in_=obuf[:, a * ca : b * ca]
                )

    nc.finalize()
    return nc


def kernel(Z_img, Z_snd, pad_idx, pad_mask, attn_dims):
    global LAST_RESULTS
    import os

    from concourse.bass_utils import run_bass_kernel_spmd

    Z_img = np.asarray(Z_img, dtype=np.float32)
    Z_snd = np.asarray(Z_snd, dtype=np.float32)
    pad_idx = np.asarray(pad_idx)
    pad_mask = np.asarray(pad_mask).astype(bool)
    A = int(attn_dims)

    B = Z_img.shape[0]
    C = Z_img.shape[1]
    CA = C - A
    G = pad_idx.shape[0]
    assert B == 16 and G % N_CORES == 0, (B, G)
    gpc = G // N_CORES

    z = Z_img.reshape(B, C, -1).mean(axis=2)
    z_img, query = z[:, :CA], z[:, CA:]

    sizes = pad_mask.sum(axis=1).astype(np.int64)
    order = np.argsort(-sizes, kind="stable")  # group ids, size descending
    caps = -(-np.maximum(sizes[order[0::N_CORES]], 1) // CHUNK) * CHUNK
    sum_caps = int(caps.sum())
    slot_off = np.concatenate([[0], np.cumsum(caps)[:-1]]).astype(np.int64)

    q_norm_max = float(np.linalg.norm(query, axis=1).max())

    # Per-core host-side layout prep.
    in_maps = []
    for c in range(N_CORES):
        keysT = np.zeros((65, sum_caps), dtype=np.float32)
        keysT[64, :] = -30000.0  # pad columns -> exp == 0 exactly
        vals = np.zeros((sum_caps, CA + 2), dtype=np.float32)
        for j in range(gpc):
            g = int(order[j * N_CORES + c])
            s = int(sizes[g])
            o = int(slot_off[j])
            if s == 0:
                # Reference yields NaN for empty groups (softmax of all
                # -inf); emit 0 instead via one fake zero-valued token.
                keysT[64, o] = 0.0
                vals[o, CA] = 1.0
                continue
            idx = pad_idx[g][pad_mask[g]]
            rows = Z_snd[idx]
            keysT[:64, o : o + s] = rows[:, CA:].T
            k_norm_max = float(np.linalg.norm(rows[:, CA:], axis=1).max())
            shift = min(q_norm_max * k_norm_max, 80.0)
            keysT[64, o : o + s] = -shift
            vals[o : o + s, :CA] = rows[:, :CA]
            vals[o : o + s, CA] = 1.0
        qT = np.empty((65, 16), dtype=np.float32)
        qT[:64] = query.T
        qT[64] = 1.0
        n_chunks = sum_caps // CHUNK
        vimg = np.ascontiguousarray(
            vals.reshape(n_chunks, CHUNK, CA + 2).transpose(1, 0, 2)
        ).reshape(CHUNK, n_chunks * (CA + 2)).astype(np.float16)
        in_maps.append({"keysT": keysT.astype(np.float16), "vals": vimg, "qT": qT.astype(np.float16)})

    nc = _build_program(caps, gpc, CA)
    trace = bool(os.environ.get("AUDIOATTN_TRACE"))
    res = run_bass_kernel_spmd(
        nc, in_maps, list(range(N_CORES)), trace=trace,
        tmpdir=os.environ.get("AUDIOATTN_TRACE_DIR") if trace else None,
    )
    LAST_RESULTS = res

    M_snd = np.empty((G, B, CA), dtype=np.float32)
    for c in range(N_CORES):
        out_c = res.results[c]["out"].astype(np.float32).reshape(B, gpc, CA)
        for j in range(gpc):
            M_snd[order[j * N_CORES + c]] = out_c[:, j]

    M_img = np.broadcast_to(z_img[None], (G, B, CA))
    return M_img, M_snd



# revision 2
# speedup vs baseline: 1.0583x; 1.0583x over previous
"""AudioAttention forward on 8 Trainium2 NeuronCores (Bass/Tile).

Reference computation (eval-mode AudioAttention):
    z      = mean_pool(Z_img)                    # [B, C]
    z_img, query = z[:, :C-A], z[:, C-A:]
    snd    = Z_snd[pad_idx]                      # [G, S, C] ragged gather
    value, key = snd[..., :C-A], snd[..., C-A:]
    scores = query @ key^T  (per group), masked softmax over S
    M_snd  = attn @ value                        # [G, B, C-A]
    M_img  = broadcast(z_img)                    # [G, B, C-A]

Sharding: groups sorted by size, dealt round-robin to 8 cores -> one
SPMD program serves all cores. Slot capacities are the per-slot max
size rounded up to 64 (not 128): token chunks of 128 may span slot
boundaries; per-slot accumulation uses partition-sliced matmuls with
bases in {0, 64} (the only bases bass accepts for K=64 operands).

DMA strategy (the previous kernel's bottleneck): each dma_start to an
SBUF tile costs one descriptor per partition line, and the two HWDGE
rings issue descriptors at ~48ns each -> descriptor COUNT, not bytes,
dominated. So: one keys DMA (65 desc), two value slices (128 desc
each), query folded into the keys array (cols 0:16), staggered 16-desc
output stores. Values travel as fp8e3 (e3m4: 4 mantissa bits) which
halves value bytes; keys/attn stay fp16 (attn in fp8 fails accuracy).

Device kernel per 128-token chunk k:
  scoresT [128,B] = matmul(lhsT=keyT_ext[65,128], rhs=qT_ext[65,B])
      row 64 of keyT_ext is (-shift) for valid tokens / -30000 for
      padding; row 64 of qT_ext is ones -> mask+shift folded into the
      contraction (exp(-30000) == 0 exactly).
  attnT = exp(scoresT)            (ACT, PSUM -> SBUF fp16)
  per slot j intersecting chunk k (rows a:b, a,b in {0,64,128}):
    m_j [B,450] += matmul(lhsT=attnT[a:b, k], rhs=val[a:b, chunk k])
      val column 448 is 1.0 for valid rows -> denominator column.
  out_j = m_j[:, :448] * reciprocal(m_j[:, 448])  (DVE/ACT alternating)
"""

import sys

if "/opt/trn_rl_repo" not in sys.path:
    sys.path.insert(0, "/opt/trn_rl_repo")

import numpy as np
import ml_dtypes

N_CORES = 8
CHUNK = 128
ALIGN = 64          # slot capacity alignment (matmul base_partition in {0,64})
VAL_FP8 = True      # values as float8e3 (e3m4); False -> fp16
N_VAL_SLICES = 2    # whole-[128]-partition value DMA slices
GEXP = 8            # chunks per exp batch
N_WARM = 7          # PE warm-up matmuls (HAM un-throttle)
OUT_SPLIT = 4       # staggered output DMA pieces

LAST_RESULTS = None  # BassKernelResults of the most recent run (for test harness)


def _plan(caps):
    """Chunk/slot piece structure. caps: per-slot capacities (mult of ALIGN,
    sum mult of CHUNK). Returns list per slot of (chunk, a, b) pieces."""
    pieces = []
    o = 0
    for cap in caps:
        sl = []
        lo = o
        while lo < o + cap:
            k = lo // CHUNK
            hi = min(o + cap, (k + 1) * CHUNK)
            sl.append((k, lo - k * CHUNK, hi - k * CHUNK))
            lo = hi
        pieces.append(sl)
        o += cap
    return pieces


def _build_program(caps, gpc, ca):
    from concourse import bacc, mybir
    from concourse.tile import TileContext

    vw = ca + 2  # value row width: features + denominator + pad-to-even
    sum_caps = int(sum(caps))
    n_chunks = sum_caps // CHUNK
    assert sum_caps % CHUNK == 0
    nc = bacc.Bacc(None, target_bir_lowering=False, debug=False)

    f32 = mybir.dt.float32
    f16 = mybir.dt.float16
    bf16 = mybir.dt.bfloat16
    vdt = mybir.dt.float8e3 if VAL_FP8 else f16
    kc = 16 + sum_caps  # query cols 0:16, then keys
    keys_d = nc.dram_tensor("keysT", [65, kc], f16, kind="ExternalInput")
    vals_d = nc.dram_tensor("vals", [CHUNK, n_chunks * vw], vdt, kind="ExternalInput")
    out_d = nc.dram_tensor("out", [16, gpc * ca], f16, kind="ExternalOutput")

    pieces = _plan(caps)

    def vsplit(n, parts):
        q, r = divmod(n, parts)
        out, a = [], 0
        for i in range(parts):
            b = a + q + (1 if i < r else 0)
            if b > a:
                out.append((a, b))
            a = b
        return out

    with TileContext(nc) as tc:
        with (
            tc.tile_pool(name="resid", bufs=1) as rpool,
            tc.tile_pool(name="recp", bufs=4) as recpool,
            tc.tile_pool(name="scps", bufs=3, space="PSUM") as scpsum,
            tc.tile_pool(name="mps", bufs=4, space="PSUM") as mpsum,
            tc.tile_pool(name="wps", bufs=1, space="PSUM") as wpsum,
        ):
            ktile = rpool.tile([65, kc], f16)
            vtile = rpool.tile([CHUNK, n_chunks * vw], vdt)
            # Keys (with folded query) first on the sync ring; value slice 0
            # concurrently on the scalar ring; value slice 1 behind keys.
            nc.sync.dma_start(out=ktile[:], in_=keys_d[:])
            vsl = vsplit(n_chunks, N_VAL_SLICES)
            engs = [nc.scalar, nc.sync]
            for i, (a, b) in enumerate(vsl):
                engs[i % 2].dma_start(
                    out=vtile[:, a * vw : b * vw], in_=vals_d[:, a * vw : b * vw]
                )
            obuf = rpool.tile([16, gpc * ca], f16)

            warm = rpool.tile([CHUNK, 512], bf16)
            nc.vector.memset(warm[:], 0.0)
            wps = wpsum.tile([CHUNK, 512], f32)
            for _ in range(N_WARM):
                nc.tensor.matmul(wps[:], warm[:, :CHUNK], warm[:], start=True, stop=True)

            # Eager scores + exp for every chunk (keys land early; attn is
            # resident so m-matmuls can consume it whenever values arrive).
            attn = rpool.tile([CHUNK, n_chunks * 16], f16)
            n_batches = -(-n_chunks // GEXP)
            for gi in range(n_batches):
                n = min(GEXP, n_chunks - gi * GEXP)
                sc = scpsum.tile([CHUNK, n * 16], f32, name=f"sc{gi}", tag="sc")
                for x in range(n):
                    t0 = 16 + (gi * GEXP + x) * CHUNK
                    nc.tensor.matmul(
                        sc[:, x * 16 : (x + 1) * 16],
                        ktile[:, t0 : t0 + CHUNK],
                        ktile[:, 0:16],
                        start=True,
                        stop=True,
                    )
                nc.scalar.activation(
                    attn[:, gi * GEXP * 16 : (gi * GEXP + n) * 16],
                    sc[:],
                    mybir.ActivationFunctionType.Exp,
                )

            out_marks = set()
            for i, (a, b) in enumerate(vsplit(gpc, OUT_SPLIT)):
                out_marks.add(b - 1)

            for j in range(gpc):
                m = mpsum.tile([16, vw], f32, name=f"m{j}", tag="m")
                sl = pieces[j]
                for pi, (k, a, b) in enumerate(sl):
                    nc.tensor.matmul(
                        m[:],
                        attn[a:b, k * 16 : (k + 1) * 16],
                        vtile[a:b, k * vw : (k + 1) * vw],
                        start=(pi == 0),
                        stop=(pi == len(sl) - 1),
                    )
                rec = recpool.tile([16, 1], f32, name=f"r{j}", tag="r")
                nc.vector.reciprocal(rec[:], m[:, ca : ca + 1])
                if j % 2 == 0:
                    nc.vector.tensor_scalar_mul(
                        obuf[:, j * ca : (j + 1) * ca], m[:, 0:ca], rec[:]
                    )
                else:
                    nc.scalar.activation(
                        obuf[:, j * ca : (j + 1) * ca],
                        m[:, 0:ca],
                        mybir.ActivationFunctionType.Copy,
                        scale=rec[:],
                    )

            for i, (a, b) in enumerate(vsplit(gpc, OUT_SPLIT)):
                engs[i % 2].dma_start(
                    out=out_d[:, a * ca : b * ca], in_=obuf[:, a * ca : b * ca]
                )

    nc.finalize()
    return nc


def kernel(Z_img, Z_snd, pad_idx, pad_mask, attn_dims):
    global LAST_RESULTS
    import os

    from concourse.bass_utils import run_bass_kernel_spmd

    Z_img = np.asarray(Z_img, dtype=np.float32)
    Z_snd = np.asarray(Z_snd, dtype=np.float32)
    pad_idx = np.asarray(pad_idx)
    pad_mask = np.asarray(pad_mask).astype(bool)
    A = int(attn_dims)

    B = Z_img.shape[0]
    C = Z_img.shape[1]
    CA = C - A
    G = pad_idx.shape[0]
    assert B == 16 and G % N_CORES == 0, (B, G)
    gpc = G // N_CORES

    z = Z_img.reshape(B, C, -1).mean(axis=2)
    z_img, query = z[:, :CA], z[:, CA:]

    sizes = pad_mask.sum(axis=1).astype(np.int64)
    order = np.argsort(-sizes, kind="stable")  # group ids, size descending
    caps = -(-np.maximum(sizes[order[0::N_CORES]], 1) // ALIGN) * ALIGN
    # total must be a multiple of CHUNK so the chunk grid tiles exactly
    if caps.sum() % CHUNK:
        caps[-1] += CHUNK - caps.sum() % CHUNK
    sum_caps = int(caps.sum())
    n_chunks = sum_caps // CHUNK
    slot_off = np.concatenate([[0], np.cumsum(caps)[:-1]]).astype(np.int64)

    q_norm_max = float(np.linalg.norm(query, axis=1).max())
    vw = CA + 2
    vdt = ml_dtypes.float8_e3m4 if VAL_FP8 else np.float16

    in_maps = []
    for c in range(N_CORES):
        keysT = np.zeros((65, 16 + sum_caps), dtype=np.float32)
        keysT[:64, 0:16] = query.T
        keysT[64, 0:16] = 1.0
        keysT[64, 16:] = -30000.0  # pad columns -> exp == 0 exactly
        vals = np.zeros((sum_caps, vw), dtype=np.float32)
        for j in range(gpc):
            g = int(order[j * N_CORES + c])
            s = int(sizes[g])
            o = int(slot_off[j])
            if s == 0:
                keysT[64, 16 + o] = 0.0
                vals[o, CA] = 1.0
                continue
            idx = pad_idx[g][pad_mask[g]]
            rows = Z_snd[idx]
            keysT[:64, 16 + o : 16 + o + s] = rows[:, CA:].T
            k_norm_max = float(np.linalg.norm(rows[:, CA:], axis=1).max())
            shift = min(q_norm_max * k_norm_max, 80.0)
            keysT[64, 16 + o : 16 + o + s] = -shift
            vals[o : o + s, :CA] = rows[:, :CA]
            vals[o : o + s, CA] = 1.0
        vimg = np.ascontiguousarray(
            vals.reshape(n_chunks, CHUNK, vw).transpose(1, 0, 2)
        ).reshape(CHUNK, n_chunks * vw).astype(vdt)
        in_maps.append({"keysT": keysT.astype(np.float16), "vals": vimg})

    nc = _build_program(caps, gpc, CA)
    trace = bool(os.environ.get("AUDIOATTN_TRACE"))
    res = run_bass_kernel_spmd(
        nc, in_maps, list(range(N_CORES)), trace=trace,
        tmpdir=os.environ.get("AUDIOATTN_TRACE_DIR") if trace else None,
    )
    LAST_RESULTS = res

    M_snd = np.empty((G, B, CA), dtype=np.float32)
    for c in range(N_CORES):
        out_c = res.results[c]["out"].astype(np.float32).reshape(B, gpc, CA)
        for j in range(gpc):
            M_snd[order[j * N_CORES + c]] = out_c[:, j]

    M_img = np.broadcast_to(z_img[None], (G, B, CA))
    return M_img, M_snd


# revision 4
# speedup vs baseline: 1.6254x; 1.5359x over previous
"""AudioAttention forward on 8 Trainium2 NeuronCores (Bass/Tile).

Reference computation (eval-mode AudioAttention):
    z      = mean_pool(Z_img)                    # [B, C]
    z_img, query = z[:, :C-A], z[:, C-A:]
    snd    = Z_snd[pad_idx]                      # [G, S, C] ragged gather
    value, key = snd[..., :C-A], snd[..., C-A:]
    scores = query @ key^T  (per group), masked softmax over S
    M_snd  = attn @ value                        # [G, B, C-A]
    M_img  = broadcast(z_img)                    # [G, B, C-A]

Sharding: groups sorted by size, dealt round-robin to 8 cores -> one
SPMD program serves all cores. Slot capacities are the per-slot max
size rounded up to 64: token chunks of 128 may span slot boundaries;
per-slot accumulation uses partition-sliced matmuls (bases in {0,64}).

DMA: each dma_start costs one descriptor per SBUF partition line and
the HWDGE ring generates descriptors serially at ~45ns each before the
doorbell, so descriptor COUNT (not bytes) sets latency. Hence: one
keys DMA (65 desc, query folded into cols 0:16), two value slices
(128 desc each), 4x16-desc output stores. Values travel as fp8e3
(e3m4) which halves value bytes; keys/attn stay fp16.

Tensor engine: the per-slot accumulation m_j[16,450] uses only 16 of
128 PE weight columns, so 4 slots run CONCURRENTLY via column tiling:
slot j accumulates at PSUM partitions 32*(j%4)..+16 of a shared
[128,450] tile (tile_position=(base, 32*(j%4)) auto-derived). One
128-lane copy evacuates 4 slots at once to SBUF. No on-device divide:
the denominator column ships with the output and the host divides.

Device kernel per 128-token chunk k:
  scoresT [128,B] = matmul(lhsT=keyT_ext[65,128], rhs=keyT_ext[:,0:16])
      row 64 of keyT_ext is (-shift) for valid tokens / -30000 for
      padding; col 0:16 row 64 is ones -> mask+shift folded into the
      contraction (exp(-30000) == 0 exactly).
  attnT = exp(scoresT)            (ACT, PSUM -> SBUF fp16)
  per slot piece (rows a:b):
    m4[32q:32q+16] += matmul(lhsT=attnT[a:b,k], rhs=val[a:b,chunk k])
      val column 448 is 1.0 for valid rows -> denominator column.
"""

import sys

if "/opt/trn_rl_repo" not in sys.path:
    sys.path.insert(0, "/opt/trn_rl_repo")

import numpy as np
import ml_dtypes

N_CORES = 8
CHUNK = 128
ALIGN = 64          # slot capacity alignment (matmul base_partition in {0,64})
VAL_FP8 = True      # values as float8e3 (e3m4); False -> fp16
GEXP = 8            # chunks per exp batch
N_WARM = 10         # PE warm-up matmuls (HAM un-throttle)
COLT = 4            # column-tiling ways (slots per PSUM round)

LAST_RESULTS = None  # BassKernelResults of the most recent run (for test harness)


def _plan(caps):
    """Per slot, list of (chunk, a, b) partition-sliced matmul pieces."""
    pieces = []
    o = 0
    for cap in caps:
        sl = []
        lo = o
        while lo < o + cap:
            k = lo // CHUNK
            hi = min(o + cap, (k + 1) * CHUNK)
            sl.append((k, lo - k * CHUNK, hi - k * CHUNK))
            lo = hi
        pieces.append(sl)
        o += cap
    return pieces


def _build_program(caps, gpc, ca):
    from concourse import bacc, mybir
    from concourse.tile import TileContext

    vw = ca + 2  # value row width: features + denominator + pad-to-even
    sum_caps = int(sum(caps))
    n_chunks = sum_caps // CHUNK
    assert sum_caps % CHUNK == 0 and gpc % COLT == 0
    rounds = gpc // COLT
    nc = bacc.Bacc(None, target_bir_lowering=False, debug=False)

    f32 = mybir.dt.float32
    f16 = mybir.dt.float16
    bf16 = mybir.dt.bfloat16
    vdt = mybir.dt.float8e3 if VAL_FP8 else f16
    kc = 16 + sum_caps  # query cols 0:16, then keys
    keys_d = nc.dram_tensor("keysT", [65, kc], f16, kind="ExternalInput")
    vals_d = nc.dram_tensor("vals", [CHUNK, n_chunks * vw], vdt, kind="ExternalInput")
    # group-major output: group q (partitions 32q..32q+16) owns slots
    # j%COLT==q, laid out round-major within the group
    out_d = nc.dram_tensor("out", [16, gpc * vw], f16, kind="ExternalOutput")

    pieces = _plan(caps)

    def vsplit(n, parts):
        q, r = divmod(n, parts)
        out, a = [], 0
        for i in range(parts):
            b = a + q + (1 if i < r else 0)
            if b > a:
                out.append((a, b))
            a = b
        return out

    with TileContext(nc) as tc:
        with (
            tc.tile_pool(name="resid", bufs=1) as rpool,
            tc.tile_pool(name="scps", bufs=3, space="PSUM") as scpsum,
            tc.tile_pool(name="mps", bufs=3, space="PSUM") as mpsum,
            tc.tile_pool(name="wps", bufs=1, space="PSUM") as wpsum,
        ):
            ktile = rpool.tile([65, kc], f16)
            vtile = rpool.tile([CHUNK, n_chunks * vw], vdt)
            # Keys first on the scalar ring (gates scores); value slice 0
            # concurrently on sync; slice 1 behind keys on scalar.
            nc.scalar.dma_start(out=ktile[:], in_=keys_d[:])
            vsl = vsplit(n_chunks, 2)
            nc.sync.dma_start(
                out=vtile[:, : vsl[0][1] * vw], in_=vals_d[:, : vsl[0][1] * vw]
            )
            nc.scalar.dma_start(
                out=vtile[:, vsl[1][0] * vw :], in_=vals_d[:, vsl[1][0] * vw :]
            )
            obuf = rpool.tile([CHUNK, rounds * vw], f16)

            warm = rpool.tile([CHUNK, 512], bf16)
            nc.vector.memset(warm[:], 0.0)
            wps = wpsum.tile([CHUNK, 512], f32)
            for _ in range(N_WARM):
                nc.tensor.matmul(wps[:], warm[:, :CHUNK], warm[:], start=True, stop=True)

            # Eager scores + exp for every chunk; attn resident in SBUF.
            attn = rpool.tile([CHUNK, n_chunks * 16], f16)
            n_batches = -(-n_chunks // GEXP)
            for gi in range(n_batches):
                n = min(GEXP, n_chunks - gi * GEXP)
                sc = scpsum.tile([CHUNK, n * 16], f32, name=f"sc{gi}", tag="sc")
                for x in range(n):
                    t0 = 16 + (gi * GEXP + x) * CHUNK
                    nc.tensor.matmul(
                        sc[:, x * 16 : (x + 1) * 16],
                        ktile[:, t0 : t0 + CHUNK],
                        ktile[:, 0:16],
                        start=True,
                        stop=True,
                    )
                nc.scalar.activation(
                    attn[:, gi * GEXP * 16 : (gi * GEXP + n) * 16],
                    sc[:],
                    mybir.ActivationFunctionType.Exp,
                )

            # Per round: COLT slots accumulate concurrently in one PSUM tile
            # (column tiling), then one 128-lane copy evacuates all of them.
            for r in range(rounds):
                m4 = mpsum.tile([CHUNK, vw], f32, name=f"m{r}", tag="m")
                for q in range(COLT):
                    j = r * COLT + q
                    sl = pieces[j]
                    for pi, (k, a, b) in enumerate(sl):
                        nc.tensor.matmul(
                            m4[32 * q : 32 * q + 16, :],
                            attn[a:b, k * 16 : (k + 1) * 16],
                            vtile[a:b, k * vw : (k + 1) * vw],
                            start=(pi == 0),
                            stop=(pi == len(sl) - 1),
                            # base partition 96 trips the auto-derive assert;
                            # positions are the operands' bases anyway
                            tile_position=(a if b - a <= 64 else 0, 32 * q),
                        )
                dst = obuf[:, r * vw : (r + 1) * vw]
                if r % 2 == 0:
                    nc.vector.tensor_copy(dst, m4[:])
                else:
                    nc.scalar.activation(
                        dst, m4[:], mybir.ActivationFunctionType.Copy
                    )

            engs = [nc.sync, nc.scalar]
            for q in range(COLT):
                engs[q % 2].dma_start(
                    out=out_d[:, q * rounds * vw : (q + 1) * rounds * vw],
                    in_=obuf[32 * q : 32 * q + 16, :],
                )

    nc.finalize()
    return nc


def kernel(Z_img, Z_snd, pad_idx, pad_mask, attn_dims):
    global LAST_RESULTS
    import os

    from concourse.bass_utils import run_bass_kernel_spmd

    Z_img = np.asarray(Z_img, dtype=np.float32)
    Z_snd = np.asarray(Z_snd, dtype=np.float32)
    pad_idx = np.asarray(pad_idx)
    pad_mask = np.asarray(pad_mask).astype(bool)
    A = int(attn_dims)

    B = Z_img.shape[0]
    C = Z_img.shape[1]
    CA = C - A
    G = pad_idx.shape[0]
    assert B == 16 and G % (N_CORES * COLT) == 0, (B, G)
    gpc = G // N_CORES
    rounds = gpc // COLT

    z = Z_img.reshape(B, C, -1).mean(axis=2)
    z_img, query = z[:, :CA], z[:, CA:]

    sizes = pad_mask.sum(axis=1).astype(np.int64)
    order = np.argsort(-sizes, kind="stable")  # group ids, size descending
    caps = -(-np.maximum(sizes[order[0::N_CORES]], 1) // ALIGN) * ALIGN
    if caps.sum() % CHUNK:
        caps[-1] += CHUNK - caps.sum() % CHUNK
    sum_caps = int(caps.sum())
    n_chunks = sum_caps // CHUNK
    slot_off = np.concatenate([[0], np.cumsum(caps)[:-1]]).astype(np.int64)

    q_norm_max = float(np.linalg.norm(query, axis=1).max())
    vw = CA + 2
    vdt = ml_dtypes.float8_e3m4 if VAL_FP8 else np.float16

    in_maps = []
    for c in range(N_CORES):
        keysT = np.zeros((65, 16 + sum_caps), dtype=np.float32)
        keysT[:64, 0:16] = query.T
        keysT[64, 0:16] = 1.0
        keysT[64, 16:] = -30000.0  # pad columns -> exp == 0 exactly
        vals = np.zeros((sum_caps, vw), dtype=np.float32)
        for j in range(gpc):
            g = int(order[j * N_CORES + c])
            s = int(sizes[g])
            o = int(slot_off[j])
            if s == 0:
                keysT[64, 16 + o] = 0.0
                vals[o, CA] = 1.0
                continue
            idx = pad_idx[g][pad_mask[g]]
            rows = Z_snd[idx]
            keysT[:64, 16 + o : 16 + o + s] = rows[:, CA:].T
            k_norm_max = float(np.linalg.norm(rows[:, CA:], axis=1).max())
            shift = min(q_norm_max * k_norm_max, 80.0)
            keysT[64, 16 + o : 16 + o + s] = -shift
            vals[o : o + s, :CA] = rows[:, :CA]
            vals[o : o + s, CA] = 1.0
        vimg = np.ascontiguousarray(
            vals.reshape(n_chunks, CHUNK, vw).transpose(1, 0, 2)
        ).reshape(CHUNK, n_chunks * vw).astype(vdt)
        in_maps.append({"keysT": keysT.astype(np.float16), "vals": vimg})

    nc = _build_program(caps, gpc, CA)
    trace = bool(os.environ.get("AUDIOATTN_TRACE"))
    res = run_bass_kernel_spmd(
        nc, in_maps, list(range(N_CORES)), trace=trace,
        tmpdir=os.environ.get("AUDIOATTN_TRACE_DIR") if trace else None,
    )
    LAST_RESULTS = res

    M_snd = np.empty((G, B, CA), dtype=np.float32)
    for c in range(N_CORES):
        # out layout: [16, (q * rounds + r) * vw + col], slot j = r*COLT + q
        out_c = (
            res.results[c]["out"].astype(np.float32).reshape(B, COLT, rounds, vw)
        )
        num = out_c[..., :CA]
        den = out_c[..., CA : CA + 1]
        mm = num / den  # [B, COLT, rounds, CA]
        for j in range(gpc):
            M_snd[order[j * N_CORES + c]] = mm[:, j % COLT, j // COLT]

    M_img = np.broadcast_to(z_img[None], (G, B, CA))
    return M_img, M_snd
